# revision 5
# baseline (speedup 1.0000x reference)
"""Trainium2 8-core kernel for nn_Attention_34402688041077 (v8).

Reference computation (fp32):
    qkv = x @ W_qkv.T + b_qkv          x:[2,2048,1024], W_qkv:[3072,1024]
    q,k,v per head (H=16, HD=64)
    attn = softmax(q k^T / sqrt(64)); out = attn v
    y = out @ W_proj.T + b_proj

Sharding (tensor parallel over heads): core c computes heads {2c, 2c+1}
for the whole batch, contributes its [128, tok] slice of pre-projection
activations to 5 token-block AllGathers, and computes y[:, 128c:128c+128]
for all tokens (output-feature-sharded projection).

Design (v5): the kernel is ACT(exp)-bound -- HW-measured exp rate is
~N/1.2 ns with negligible per-instruction overhead, so the exp floor is
B*H_pc*N^2/core = 16.8M elems ~= 109 us.  Everything else hides under it:
  - 512-wide q-blocks process BOTH heads per kc-tile; the two heads'
    score matmuls use disjoint PE row-groups (head0 partitions 0-63,
    head1 64-127, tile_position auto-derived) and run CONCURRENTLY,
    halving score PE time.
  - attn@v keeps the ones-column trick (lhsT [128, 65]; row sums fall
    out of the same matmul, normalization by reciprocal-multiply).
  - All dtypes bf16 (fp32 PSUM).  fp8 anywhere in the attention path
    measurably costs ~2e-2 rel err (the attention output is an average:
    per-element noise passes ~1:1 to the output), so it is not used.
  - QKV(b1) + projection half-blocks run as fill tasks in PE slack
    between attention kc-iterations; input DMAs split across both HWDGE
    rings; block tails (last av + normalization) are deferred into the
    next block so the in-order PE stream never starves ACT.

  - AllGathers have a ~15us fixed cost on this fabric and serialize on
    the collective ring, so there are only THREE per pass (b0 full, b1
    halves), each fired where it overlaps a full attention block; the
    final gather + its projections are CARRIED into the next rep's
    stream (reps pipeline; the per-rep boundary exposes only ~7us).

Measured rel err vs fp32 ref ~5e-3.
"""

import sys

sys.path.insert(0, "/opt/trn_rl_repo")

import numpy as np
import ml_dtypes

import concourse.bass as bass  # noqa: F401  (registers engine types)
import concourse.tile as tile
from concourse import bacc, mybir
from concourse.bass_utils import run_bass_kernel_spmd
from concourse.masks import make_identity

BF16_NP = ml_dtypes.bfloat16
F32 = mybir.dt.float32
BF16 = mybir.dt.bfloat16

N_CORES = 8
B, N, DIM, H, HD = 2, 2048, 1024, 16, 64
T = B * N                # 4096 flattened tokens
HPC = H // N_CORES       # 2 heads per core
FPC = HPC * HD           # 128 features per core
SCALE = 1.0 / np.sqrt(HD)
TPB = N // 128           # 16 k token-tiles per batch
QC = N // 512            # 4 q-chunks per batch
AV_LAG = 4               # attn@v lags scores by this many kc-iterations

_NC_CACHE = {}


def _make_pools(nc, tc, ctx):
    """Open all pools once (shared across reps -- tile tags make slots
    persistent; cross-rep WAR hazards are handled by the tile deps)."""
    P = {}
    P["slabs"] = ctx.enter_context(tc.tile_pool(name="slabs", bufs=1))
    P["psSC"] = ctx.enter_context(
        tc.tile_pool(name="psSC", bufs=2, space="PSUM"))
    P["psAO"] = ctx.enter_context(
        tc.tile_pool(name="psAO", bufs=1, space="PSUM"))
    P["psF"] = ctx.enter_context(
        tc.tile_pool(name="psF", bufs=2, space="PSUM"))
    P["attnp"] = ctx.enter_context(tc.tile_pool(name="attnp", bufs=8))
    P["vtp"] = ctx.enter_context(tc.tile_pool(name="vtp", bufs=3))
    P["aoraw"] = ctx.enter_context(tc.tile_pool(name="aoraw", bufs=4))
    P["normp"] = ctx.enter_context(tc.tile_pool(name="normp", bufs=3))
    P["rhp"] = ctx.enter_context(tc.tile_pool(name="rhp", bufs=4))
    P["yp"] = ctx.enter_context(tc.tile_pool(name="yp", bufs=2))
    P["dramloc"] = ctx.enter_context(
        tc.tile_pool(name="dramloc", bufs=2, space="DRAM"))
    slabs = P["slabs"]
    # persistent per-batch slabs, allocated once and shared by all reps:
    # q/k transposed [feat, tok] (rows 0-63 head 0, 64-127 head 1);
    # v_ext [tok%128, tok_tile, head, HD+1] with a ones column at HD so
    # attn@v also yields softmax row sums.
    P["qsl"] = [slabs.tile([128, N], BF16, tag=f"qsl{b}", name=f"qsl{b}")
                for b in range(B)]
    P["ksl"] = [slabs.tile([128, N], BF16, tag=f"ksl{b}", name=f"ksl{b}")
                for b in range(B)]
    P["v_ext"] = [slabs.tile([128, TPB, HPC, HD + 1], BF16, tag=f"vext{b}",
                             name=f"vext{b}") for b in range(B)]
    P["aosl"] = [[slabs.tile([HD, N], BF16, tag=f"ao{b}{h}",
                             name=f"ao{b}{h}") for h in range(HPC)]
                 for b in range(B)]
    for b in range(B):
        # whole-tile memset: the ones column stays 1.0, the rest is
        # overwritten by the v stores
        nc.gpsimd.memset(P["v_ext"][b][:], 1.0)
    return P


# gather/projection blocks: AllGathers have a large fixed cost and
# serialize on the collective ring, so only three per pass, each fired
# right after the last attention block covering its columns
GBLK = [(0, 0, 2048), (1, 0, 1024), (1, 1024, 1024)]


def _body(nc, tc, P, xT_sb, w_sb, wp_sb, bqk_sb, bp_sb, dram, y, rep=0,
          collective=True, ident=None, carry_in=None, last=True):
    """One full forward pass for this core's shard.

    xT_sb: [kc][bb] -> [128, 2048] bf16 tiles of x^T (dim-chunk, batch)
    carry_in: closures (final gather + projections) deferred from the
    previous rep, emitted early in this rep's stream.  Returns this
    rep's carry (or emits it inline when last=True).
    """
    EXP = mybir.ActivationFunctionType.Exp
    if True:
        psSC, psAO, psF = P["psSC"], P["psAO"], P["psF"]
        attnp, vtp, aoraw = P["attnp"], P["vtp"], P["aoraw"]
        normp, rhp, yp, dramloc = P["normp"], P["rhp"], P["yp"], P["dramloc"]
        qsl, ksl, v_ext, aosl = P["qsl"], P["ksl"], P["v_ext"], P["aosl"]

        ag_in = [dramloc.tile([FPC, w], BF16, name=f"agin{i}")
                 for i, (_, _, w) in enumerate(GBLK)]
        ag_out = [dram.tile([DIM, w], BF16, addr_space="Shared",
                            name=f"agout{rep}_{i}") for i, (_, _, w) in
                  enumerate(GBLK)]

        def qkv_mm_half(ps, ft, bb, tcb, half):
            # half 0: kc 0-3 (start), half 1: kc 4-7 (stop)
            for kc in range(4 * half, 4 * half + 4):
                nc.tensor.matmul(
                    ps[:],
                    lhsT=w_sb[kc][:, ft * 128:(ft + 1) * 128],
                    rhs=xT_sb[kc][bb][:, tcb * 512:(tcb + 1) * 512],
                    start=(kc == 0), stop=(kc == 7),
                )

        def qkv_qk_chunk(bb, ft, tcb):
            # ft: 0 = q, 1 = k -- one unsplit chunk (lead-in path)
            dst = (qsl if ft == 0 else ksl)[bb]
            ps = psF.tile([128, 512], F32, tag="fill", name="psqk")
            qkv_mm_half(ps, ft, bb, tcb, 0)
            qkv_mm_half(ps, ft, bb, tcb, 1)
            nc.vector.tensor_scalar_add(
                dst[:, tcb * 512:(tcb + 1) * 512], ps[:],
                bqk_sb[:, ft:ft + 1])

        def qkv_qk_tasks(bb, ft, tcb):
            # same split into two fill tasks sharing one PSUM tile
            dst = (qsl if ft == 0 else ksl)[bb]
            st = {}

            def run_a():
                st["ps"] = psF.tile([128, 512], F32, tag="fill", name="psqk")
                qkv_mm_half(st["ps"], ft, bb, tcb, 0)

            def run_b():
                qkv_mm_half(st["ps"], ft, bb, tcb, 1)
                nc.vector.tensor_scalar_add(
                    dst[:, tcb * 512:(tcb + 1) * 512], st["ps"][:],
                    bqk_sb[:, ft:ft + 1])

            return [{"run": run_a}, {"run": run_b}]

        def v_store(bb, tcb, vt, jj):
            a = tcb * 4 + jj              # token-tile index 0..15
            tp = psF.tile([128, 128], BF16, tag="fill", name="tp")
            nc.tensor.transpose(
                tp[:], vt[:, jj * 128:(jj + 1) * 128], ident[:])
            # one strided copy covers both heads' 64-feature slices
            nc.vector.tensor_copy(
                v_ext[bb][:, a, :, 0:HD],
                tp[:].rearrange("p (h f) -> p h f", h=2))

        def qkv_v_chunk(bb, tcb):
            # v computed weight-stationary as v^T, PE-transposed back to
            # [token, feat] into the extended slab
            ps = psF.tile([128, 512], F32, tag="fill", name="psvt")
            qkv_mm_half(ps, 2, bb, tcb, 0)
            qkv_mm_half(ps, 2, bb, tcb, 1)
            vt = vtp.tile([128, 512], BF16, tag="vt", name="vt")
            nc.vector.tensor_copy(vt[:], ps[:])
            for jj in range(4):
                v_store(bb, tcb, vt, jj)

        def qkv_v_tasks(bb, tcb):
            st = {}

            def run_a():
                st["ps"] = psF.tile([128, 512], F32, tag="fill", name="psvt")
                qkv_mm_half(st["ps"], 2, bb, tcb, 0)
                qkv_mm_half(st["ps"], 2, bb, tcb, 1)
                st["vt"] = vtp.tile([128, 512], BF16, tag="vt", name="vt")
                nc.vector.tensor_copy(st["vt"][:], st["ps"][:])

            def run_b():
                for jj in range(4):
                    v_store(bb, tcb, st["vt"], jj)

            return [{"run": run_a}, {"run": run_b}]

        # ---- fill task machinery: deferred PE-light work emitted between
        # attention kc-iterations.  Tasks are {pre, run}: `pre` issues
        # input DMAs (no PE time) ahead of `run` so matmuls never
        # head-of-line block the PE stream on a DMA.
        fill_q = []
        fill_state = {"pre": 0, "run": 0, "tick": 0}

        def _advance_pre():
            while fill_state["pre"] < min(fill_state["run"] + 2, len(fill_q)):
                pre = fill_q[fill_state["pre"]].get("pre")
                if pre:
                    pre()
                fill_state["pre"] += 1

        def fill_tick(period):
            fill_state["tick"] += 1
            if fill_state["tick"] % period:
                return
            _advance_pre()
            if fill_state["run"] < len(fill_q):
                t = fill_q[fill_state["run"]]
                # a task with input DMAs never runs at the same tick its
                # prefetch was issued -- the DMA needs a period of lead
                if t.get("pre") is None or fill_state["pre"] > \
                        fill_state["run"] + 1:
                    t["run"]()
                    fill_state["run"] += 1

        def drain_fill():
            while fill_state["run"] < len(fill_q):
                _advance_pre()
                fill_q[fill_state["run"]]["run"]()
                fill_state["run"] += 1

        def proj_half(tb, o):
            bb, col0, w = GBLK[tb]
            wo = min(512, w - o)
            rts = []

            def pre():
                for g in range(2):  # two 4-wide merged rhs DMAs
                    rt = rhp.tile([128, 4, 512], BF16, tag="agr", name="agr")
                    nc.sync.dma_start(
                        out=rt[:, :, 0:wo],
                        in_=ag_out[tb][g * 512:(g + 1) * 512,
                                       o:o + wo].rearrange(
                            "(j p) t -> p j t", p=128))
                    rts.append(rt)

            def run():
                if not rts:
                    pre()
                ps = psF.tile([128, 512], F32, tag="fill", name="psp")
                for g in range(2):
                    for j in range(4):
                        nc.tensor.matmul(
                            ps[:, 0:wo], lhsT=wp_sb[g * 4 + j][:],
                            rhs=rts[g][:, j, 0:wo],
                            start=(g == 0 and j == 0),
                            stop=(g == 1 and j == 3))
                ysb = yp.tile([128, 512], F32, tag="ysb", name="ysb")
                nc.vector.tensor_scalar_add(ysb[:, 0:wo], ps[:, 0:wo],
                                            bp_sb[:])
                c0 = bb * N + col0 + o
                nc.sync.dma_start(out=y[:, c0:c0 + wo], in_=ysb[:, 0:wo])

            return {"pre": pre, "run": run}

        def gather_block(tb):
            bb, col0, w = GBLK[tb]
            for h in range(HPC):
                nc.sync.dma_start(
                    out=ag_in[tb][h * HD:(h + 1) * HD, :],
                    in_=aosl[bb][h][:, col0:col0 + w])
            if collective:
                nc.gpsimd.collective_compute(
                    "AllGather", mybir.AluOpType.bypass,
                    replica_groups=[list(range(N_CORES))],
                    ins=[ag_in[tb][:].opt()], outs=[ag_out[tb][:].opt()],
                )
            else:  # timing-sim variant: token dep so proj waits on attn
                nc.gpsimd.dma_start(out=ag_out[tb][0:1, 0:128],
                                    in_=ag_in[tb][0:1, 0:128])

        def norm_head(bb, h, qc, ao_ps):
            # copy out of PSUM early; reciprocal-multiply from SBUF.
            # partition_broadcast needs its source at base partition 0, so
            # the sums row goes through a partition-0 tile on gpsimd.
            ar = aoraw.tile([HD + 1, 512], F32, tag="ar", name="ar")
            nc.vector.tensor_copy(ar[:], ao_ps[:])
            srow = normp.tile([1, 512], F32, tag="srow", name="srow")
            nc.gpsimd.tensor_copy(srow[:], ar[HD:HD + 1, :])
            bc = normp.tile([HD, 512], F32, tag="bc", name="bc")
            nc.gpsimd.partition_broadcast(bc[:], srow[:])
            rec = normp.tile([HD, 512], F32, tag="rec", name="rec")
            nc.vector.reciprocal(rec[:], bc[:])
            nc.vector.tensor_mul(
                aosl[bb][h][:, qc * 512:(qc + 1) * 512], ar[0:HD, :], rec[:])

        def attn_block(bb, qc, prev_tail=None, after_prev_tail=None,
                       fill_period=2):
            # 512-wide q-chunk, BOTH heads per kc-iteration: the two score
            # matmuls write the two 512-halves (= the two banks) of ONE
            # PSUM tile from disjoint PE row groups (partitions 0-63 /
            # 64-127 via auto tile_position), so they run concurrently AND
            # a single N=1024 exp covers both heads (HW-measured exp is
            # ~128ns/instr + 0.78ns/col, so big tiles win).  attn@v lags
            # scores by AV_LAG iterations; the last avs and the
            # normalization are deferred into the next block.
            colq = qc * 512
            ao = [psAO.tile([HD + 1, 512], F32, tag=f"ao{h}",
                            name=f"ao{h}") for h in range(HPC)]
            ats = [None] * TPB

            def av(kc):
                for h in range(HPC):
                    nc.tensor.matmul(
                        ao[h][:], lhsT=v_ext[bb][:, kc, h, :],
                        rhs=ats[kc][:, h, :],
                        start=(kc == 0), stop=(kc == TPB - 1))

            for kc in range(TPB):
                sc = psSC.tile([128, 2, 512], F32, tag="sc", name="sc")
                for h in range(HPC):
                    nc.tensor.matmul(
                        sc[:, h, :],
                        lhsT=ksl[bb][h * HD:(h + 1) * HD,
                                     kc * 128:(kc + 1) * 128],
                        rhs=qsl[bb][h * HD:(h + 1) * HD,
                                    colq:colq + 512],
                        start=True, stop=True)
                at = attnp.tile([128, 2, 512], BF16, tag="at", name="at")
                nc.scalar.activation(out=at[:], in_=sc[:], func=EXP)
                ats[kc] = at
                if kc == 0 and prev_tail is not None:
                    prev_tail()
                    if after_prev_tail is not None:
                        after_prev_tail()
                if kc >= AV_LAG:
                    av(kc - AV_LAG)
                fill_tick(fill_period)

            def tail():
                for kc in range(TPB - AV_LAG, TPB):
                    av(kc)
                for h in range(HPC):
                    norm_head(bb, h, qc, ao[h])

            return tail

        # ---- emission order = engine-stream order = schedule ----
        # in-stream lead-in: just what the first block needs early; the
        # rest of b0's QKV and all of b1's become fill work.
        qkv_qk_chunk(0, 1, 0)                # k tokens 0:512 (kc 0-3)
        qkv_qk_chunk(0, 0, 0)                # q cols 0:512
        qkv_qk_chunk(0, 1, 1)                # k tokens 512:1024
        qkv_v_chunk(0, 0)                    # v token-tiles 0-3

        # deferred tail of the PREVIOUS rep: its last gather fires now
        # (the ring does it under our lead-in); its projections become
        # the first fill tasks
        if carry_in is not None:
            carry_in["gather"]()
            for t in carry_in["projs"]:
                fill_q.append(t)

        qkv_qk_chunk(0, 1, 2)
        qkv_qk_chunk(0, 1, 3)

        fill_q.extend(qkv_v_tasks(0, 1))     # v tiles 4-7 (needed iter ~6)
        fill_q.extend(qkv_v_tasks(0, 2))
        fill_q.extend(qkv_v_tasks(0, 3))
        fill_q.extend(qkv_qk_tasks(0, 0, 1))  # q cols 512:1024 (block 2)
        fill_q.extend(qkv_qk_tasks(0, 0, 2))
        fill_q.extend(qkv_qk_tasks(0, 0, 3))

        tail = attn_block(0, 0, fill_period=1)
        tail = attn_block(0, 1, prev_tail=tail, fill_period=1)
        tail = attn_block(0, 2, prev_tail=tail, fill_period=1)
        # b1's k chunks have no consumers until the b1 blocks, so they
        # ride the last b0 block's fill slots; q cols 512:2048 likewise
        # ride block (1,0)'s slots (consumed one block later).  Only
        # q cols 0:512 and v must be serial at the batch boundary.
        for tcb in range(4):
            fill_q.extend(qkv_qk_tasks(1, 1, tcb))
        tail = attn_block(0, 3, prev_tail=tail, fill_period=2)
        drain_fill()
        qkv_qk_chunk(1, 0, 0)
        for tcb in range(4):
            qkv_v_chunk(1, tcb)
        for tcb in range(1, 4):
            fill_q.extend(qkv_qk_tasks(1, 0, tcb))
        # b0 attention done after (0,3)'s deferred tail inside (1,0):
        # gather0 = all of b0 (4MB out), a full batch of overlap time
        tail = attn_block(1, 0, prev_tail=tail,
                          after_prev_tail=lambda: gather_block(0),
                          fill_period=2)
        fill_q.append(proj_half(0, 0))       # b0 projections
        fill_q.append(proj_half(0, 512))
        fill_q.append(proj_half(0, 1024))
        fill_q.append(proj_half(0, 1536))
        tail = attn_block(1, 1, prev_tail=tail, fill_period=3)
        tail = attn_block(1, 2, prev_tail=tail,
                          after_prev_tail=lambda: gather_block(1),
                          fill_period=3)
        fill_q.append(proj_half(1, 0))
        fill_q.append(proj_half(1, 512))
        tail = attn_block(1, 3, prev_tail=tail, fill_period=3)
        drain_fill()
        tail()

        def carry_gather():
            gather_block(2)

        carry_projs = [proj_half(2, 0), proj_half(2, 512)]
        if last:
            carry_gather()
            for t in carry_projs:
                t["pre"]()
            for t in carry_projs:
                t["run"]()
            return None
        return {"gather": carry_gather, "projs": carry_projs}


def _build(reps=1, collective=True, num_devices=N_CORES):
    nc = bacc.Bacc("TRN2", target_bir_lowering=False, debug=False,
                   num_devices=num_devices)
    # inputs are host-pre-tiled so every DMA reads one contiguous block
    xT = nc.dram_tensor("xT", [B, 8, 128, N], BF16,
                        kind="ExternalInput").ap()      # [bb, kc, p, tok]
    wqkvT = nc.dram_tensor("wqkvT", [128, 8, 3 * FPC], BF16,
                           kind="ExternalInput").ap()   # [p, kc, feat]
    bqk = nc.dram_tensor("bqk", [2, FPC, 1], F32, kind="ExternalInput").ap()
    wpT = nc.dram_tensor("wpT", [128, 8, FPC], BF16,
                         kind="ExternalInput").ap()     # [p, kc, fo]
    bp = nc.dram_tensor("bp", [FPC, 1], F32, kind="ExternalInput").ap()
    y = nc.dram_tensor("y", [FPC, T], F32, kind="ExternalOutput").ap()

    from contextlib import ExitStack
    with tile.TileContext(nc) as tc, ExitStack() as ctx:
        if True:
            const = ctx.enter_context(tc.tile_pool(name="const", bufs=1))
            dram = ctx.enter_context(
                tc.tile_pool(name="dram", bufs=1, space="DRAM"))
            P = _make_pools(nc, tc, ctx)
            xT_sb = [[None] * B for _ in range(8)]  # [kc][bb] -> [128, N]
            # lead-in is x(b0)-DMA-bound: split batch-0 x tiles across
            # BOTH HWDGE rings (sync + scalar queues); QKV weights on the
            # scalar ring ahead of its share of x tiles
            w_all = const.tile([128, 8, 3 * FPC], BF16, tag="w", name="w_all")
            nc.scalar.dma_start(out=w_all[:], in_=wqkvT[:])
            w_sb = [w_all[:, kc, :] for kc in range(8)]
            bqk_sb = const.tile([FPC, 2], F32, tag="bqk", name="bqk_sb")
            nc.sync.dma_start(out=bqk_sb[:, 0:1], in_=bqk[0])
            nc.sync.dma_start(out=bqk_sb[:, 1:2], in_=bqk[1])
            for kc in range(8):
                for bb in range(B):
                    xT_sb[kc][bb] = const.tile(
                        [128, N], BF16, tag=f"xT{kc}_{bb}",
                        name=f"xT{kc}_{bb}")
            for kc in range(8):
                eng = nc.sync if kc % 2 == 0 else nc.scalar
                eng.dma_start(out=xT_sb[kc][0][:], in_=xT[0, kc])
            for kc in range(8):
                eng = nc.sync if kc % 2 == 0 else nc.scalar
                eng.dma_start(out=xT_sb[kc][1][:], in_=xT[1, kc])
            # proj weights are needed late; lowest DMA priority
            wp_all = const.tile([128, 8, FPC], BF16, tag="wp", name="wp_all")
            nc.sync.dma_start(out=wp_all[:], in_=wpT[:])
            wp_sb = [wp_all[:, kc, :] for kc in range(8)]
            bp_sb = const.tile([FPC, 1], F32, tag="bp", name="bp_sb")
            nc.sync.dma_start(out=bp_sb[:], in_=bp[:])
            ident = const.tile([128, 128], BF16, tag="ident", name="ident")
            make_identity(nc, ident[:])
            # ACT exp-table warm-up pulls the one-time ~2.7us
            # ACT_TABLE_LOAD off the first real attention exp; output
            # lands in y[0:1, 0:8] which projections later overwrite.
            warm = const.tile([1, 8], F32, tag="warm", name="warm")
            nc.gpsimd.memset(warm[:], 0.0)
            warm2 = const.tile([1, 8], F32, tag="warm2", name="warm2")
            nc.scalar.activation(out=warm2[:], in_=warm[:],
                                 func=mybir.ActivationFunctionType.Exp)
            nc.sync.dma_start(out=y[0:1, 0:8], in_=warm2[:])

            carry = None
            for r in range(reps):
                carry = _body(nc, tc, P, xT_sb, w_sb, wp_sb, bqk_sb, bp_sb,
                              dram, y, rep=r, collective=collective,
                              ident=ident, carry_in=carry,
                              last=(r == reps - 1))
    nc.compile()
    return nc


def _prepare_in_maps(x, W_qkv, b_qkv, W_proj, b_proj):
    x = np.asarray(x, dtype=np.float32)
    W_qkv = np.asarray(W_qkv, dtype=np.float32)
    b_qkv = np.asarray(b_qkv, dtype=np.float32)
    W_proj = np.asarray(W_proj, dtype=np.float32)
    b_proj = np.asarray(b_proj, dtype=np.float32)

    xT = np.ascontiguousarray(x.reshape(T, DIM).T).astype(BF16_NP)
    # pre-tile to [bb, kc, 128, N] so device DMAs are contiguous blocks
    xT = np.ascontiguousarray(
        xT.reshape(8, 128, B, N).transpose(2, 0, 1, 3))
    # v bias folds through attention (softmax rows sum to 1) into the
    # projection bias: y += b_v @ W_proj.T
    bv = b_qkv[2 * DIM:3 * DIM]
    bp_eff = b_proj + bv @ W_proj.T

    in_maps = []
    for c in range(N_CORES):
        r0 = c * FPC
        wq = W_qkv[r0:r0 + FPC] * SCALE            # fold 1/sqrt(HD) into q
        wk = W_qkv[DIM + r0:DIM + r0 + FPC]
        wv = W_qkv[2 * DIM + r0:2 * DIM + r0 + FPC]
        wqkvT = np.ascontiguousarray(
            np.concatenate([wq, wk, wv], axis=0).T).astype(BF16_NP)
        wqkvT = np.ascontiguousarray(
            wqkvT.reshape(8, 128, 3 * FPC).transpose(1, 0, 2))
        bqk = np.stack([b_qkv[r0:r0 + FPC] * SCALE,
                        b_qkv[DIM + r0:DIM + r0 + FPC]])[:, :, None]
        wpT = np.ascontiguousarray(W_proj[r0:r0 + FPC].T).astype(BF16_NP)
        wpT = np.ascontiguousarray(
            wpT.reshape(8, 128, FPC).transpose(1, 0, 2))
        bp = bp_eff[r0:r0 + FPC][:, None]
        in_maps.append({
            "xT": xT,
            "wqkvT": wqkvT,
            "bqk": np.ascontiguousarray(bqk, dtype=np.float32),
            "wpT": wpT,
            "bp": np.ascontiguousarray(bp, dtype=np.float32),
        })
    return in_maps


def _assemble(results):
    # per-core y is [128, T] = (this core's 128 output features) x tokens
    cols = [np.asarray(results[c]["y"], dtype=np.float32).T
            for c in range(N_CORES)]
    return np.concatenate(cols, axis=1).reshape(B, N, DIM)


def kernel(x, W_qkv, b_qkv, W_proj, b_proj):
    if "nc" not in _NC_CACHE:
        _NC_CACHE["nc"] = _build()
    nc = _NC_CACHE["nc"]
    in_maps = _prepare_in_maps(x, W_qkv, b_qkv, W_proj, b_proj)
    res = run_bass_kernel_spmd(nc, in_maps, core_ids=list(range(N_CORES)))
    return _assemble(res.results)
